# revision 1
# baseline (speedup 1.0000x reference)
"""Pairwise cosine similarity [8192, 8192] on 8 Trainium2 NeuronCores.

out[n, m] = dot(input1[n], input2[m]) / max(||input1[n]|| * ||input2[m]||, eps)

Sharding: rows of input1 (N) are split across the 8 cores; input2 is
replicated. Each core computes a [1024, 8192] slab of the output.

Device kernel (per core), with D = 512 contraction dim:
  - Inputs are fed host-transposed as x1t [512, 1024] and x2t [512, 8192]
    (d-major), cast to fp16, so the TensorE contraction needs no on-chip
    transposes.
  - Row norms are computed on-chip: ACT squares each [128, 512] chunk, a
    ones-stationary matmul reduces over the partition (d) axis, giving the
    squared norms replicated across all 128 partitions; then
    reciprocal (DVE) + sqrt (ACT) produce 1/norm, still replicated.
  - Both operand matrices are pre-scaled by their 1/norm (DVE), so the main
    matmul directly produces the cosine similarities.
  - Main matmul: 8 m-tiles x 16 n-chunks x 4 k-tiles of
    [128,128] x [128,512] fp16 MMs accumulating in PSUM (fp32), copied
    PSUM->SBUF (alternating ACT/DVE) and DMAed to DRAM.

eps note: inputs are randn(512)-distributed, so every norm is ~22.6 and the
max(., eps=1e-8) in the reference never binds; the kernel divides directly.
"""

import os
import sys

import numpy as np

sys.path.insert(0, "/opt/trn_rl_repo")

import concourse.bass as bass  # noqa: E402
import concourse.mybir as mybir  # noqa: E402
from concourse import bacc  # noqa: E402
from concourse.tile import TileContext  # noqa: E402
from concourse.bass_utils import run_bass_kernel_spmd  # noqa: E402

N_CORES = 8
N = 8192  # rows of input1 (output rows)
M = 8192  # rows of input2 (output cols)
D = 512  # feature dim (contraction)
N_SHARD = N // N_CORES  # 1024 rows per core

P = 128  # partitions
CHUNK = 512  # free-dim chunk (= max fp32 PSUM bank free size)
KT = D // P  # 4 k-tiles
M_TILES = N_SHARD // P  # 8 output row tiles per core
N_CHUNKS = M // CHUNK  # 16 output col chunks
X1_CHUNKS = N_SHARD // CHUNK  # 2 chunks of x1 free dim

DT = mybir.dt.float16
NP_DT = np.float16

_CACHE = {}


def _build():
    """Build + bacc-compile the per-core Bass kernel (SPMD: same program
    on all cores, different DRAM contents)."""
    nc = bacc.Bacc("TRN2", target_bir_lowering=False, debug=False)

    x1t = nc.dram_tensor("x1t", [D, N_SHARD], DT, kind="ExternalInput")
    x2t = nc.dram_tensor("x2t", [D, M], DT, kind="ExternalInput")
    out_d = nc.dram_tensor("out", [N_SHARD, M], mybir.dt.float32, kind="ExternalOutput")

    with TileContext(nc) as tc:
        with (
            tc.tile_pool(name="consts", bufs=1) as consts,
            tc.tile_pool(name="x1raw", bufs=KT * X1_CHUNKS) as x1raw_pool,
            tc.tile_pool(name="x2raw", bufs=KT * N_CHUNKS) as x2raw_pool,
            tc.tile_pool(name="x1n", bufs=KT * X1_CHUNKS) as x1n_pool,
            tc.tile_pool(name="x2n", bufs=KT * N_CHUNKS) as x2n_pool,
            tc.tile_pool(name="inv1", bufs=X1_CHUNKS) as inv1_pool,
            tc.tile_pool(name="inv2", bufs=N_CHUNKS) as inv2_pool,
            tc.tile_pool(name="sq", bufs=6) as sq_pool,
            tc.tile_pool(name="rtmp", bufs=4) as rtmp_pool,
            tc.tile_pool(name="outsb", bufs=6) as out_pool,
            tc.tile_pool(name="pnorm", bufs=2, space="PSUM") as pnorm_pool,
            tc.tile_pool(name="pmain", bufs=6, space="PSUM") as pmain_pool,
        ):
            ones = consts.tile([P, P], DT)
            nc.vector.memset(ones[:], 1.0)

            # ---- loads (chunk-granular so consumers can start early) ----
            x1raw = {}
            for k in range(KT):
                for c in range(X1_CHUNKS):
                    t = x1raw_pool.tile([P, CHUNK], DT, tag="x1raw")
                    nc.sync.dma_start(
                        out=t[:],
                        in_=x1t[k * P : (k + 1) * P, c * CHUNK : (c + 1) * CHUNK],
                    )
                    x1raw[(k, c)] = t
            x2raw = {}
            for k in range(KT):
                for c in range(N_CHUNKS):
                    t = x2raw_pool.tile([P, CHUNK], DT, tag="x2raw")
                    nc.sync.dma_start(
                        out=t[:],
                        in_=x2t[k * P : (k + 1) * P, c * CHUNK : (c + 1) * CHUNK],
                    )
                    x2raw[(k, c)] = t

            def norm_and_scale(raw, n_chunks, inv_pool, n_pool, tag):
                """Per 512-wide chunk: 1/||col|| (replicated over partitions)
                then pre-scale the raw chunk tiles by it."""
                inv = {}
                scaled = {}
                for c in range(n_chunks):
                    ps = pnorm_pool.tile([P, CHUNK], mybir.dt.float32, tag="pnorm")
                    for k in range(KT):
                        sq = sq_pool.tile([P, CHUNK], DT, tag="sq")
                        nc.scalar.square(sq[:], raw[(k, c)][:])
                        nc.tensor.matmul(
                            ps[:], ones[:], sq[:], start=(k == 0), stop=(k == KT - 1)
                        )
                    rt = rtmp_pool.tile([P, CHUNK], mybir.dt.float32, tag="rtmp")
                    nc.vector.reciprocal(rt[:], ps[:])
                    iv = inv_pool.tile([P, CHUNK], DT, tag=tag + "inv")
                    nc.scalar.sqrt(iv[:], rt[:])
                    inv[c] = iv
                    for k in range(KT):
                        sc = n_pool.tile([P, CHUNK], DT, tag=tag + "n")
                        nc.vector.tensor_mul(sc[:], raw[(k, c)][:], iv[:])
                        scaled[(k, c)] = sc
                return inv, scaled

            _, x1n = norm_and_scale(x1raw, X1_CHUNKS, inv1_pool, x1n_pool, "x1")
            _, x2n = norm_and_scale(x2raw, N_CHUNKS, inv2_pool, x2n_pool, "x2")

            # ---- main matmul + epilogue ----
            for c in range(N_CHUNKS):
                for m in range(M_TILES):
                    c1, j = divmod(m, 4)  # x1n chunk + 128-slice within it
                    ps = pmain_pool.tile([P, CHUNK], mybir.dt.float32, tag="pmain")
                    for k in range(KT):
                        nc.tensor.matmul(
                            ps[:],
                            x1n[(k, c1)][:, j * P : (j + 1) * P],
                            x2n[(k, c)][:],
                            start=(k == 0),
                            stop=(k == KT - 1),
                        )
                    ob = out_pool.tile([P, CHUNK], mybir.dt.float32, tag="outsb")
                    if (c * M_TILES + m) % 2 == 0:
                        nc.scalar.copy(ob[:], ps[:])
                    else:
                        nc.vector.tensor_copy(ob[:], ps[:])
                    nc.sync.dma_start(
                        out=out_d[m * P : (m + 1) * P, c * CHUNK : (c + 1) * CHUNK],
                        in_=ob[:],
                    )

    nc.compile()
    return nc


def _get_nc():
    if "nc" not in _CACHE:
        _CACHE["nc"] = _build()
    return _CACHE["nc"]


def _prep_in_maps(input1, input2):
    input1 = np.asarray(input1, dtype=np.float32)
    input2 = np.asarray(input2, dtype=np.float32)
    assert input1.shape == (N, D) and input2.shape == (M, D)
    x2t = np.ascontiguousarray(input2.T).astype(NP_DT)
    in_maps = []
    for c in range(N_CORES):
        sl = input1[c * N_SHARD : (c + 1) * N_SHARD]
        x1t = np.ascontiguousarray(sl.T).astype(NP_DT)
        in_maps.append({"x1t": x1t, "x2t": x2t})
    return in_maps


def _run(input1, input2, trace=False, trace_kwargs=None):
    nc = _get_nc()
    in_maps = _prep_in_maps(input1, input2)
    res = run_bass_kernel_spmd(
        nc, in_maps, list(range(N_CORES)), trace=trace, **(trace_kwargs or {})
    )
    out = np.concatenate([res.results[i]["out"] for i in range(N_CORES)], axis=0)
    return out, res


def kernel(input1, input2):
    out, _ = _run(input1, input2, trace=False)
    return out


# revision 2
# speedup vs baseline: 1.1985x; 1.1985x over previous
"""Pairwise cosine similarity [8192, 8192] on 8 Trainium2 NeuronCores.

out[n, m] = dot(input1[n], input2[m]) / max(||input1[n]|| * ||input2[m]||, eps)

Sharding: rows of input1 (N) are split across the 8 cores; input2 is
replicated. Each core computes a [1024, 8192] slab of the output.

Device kernel (per core), D = 512 contraction dim:
  - Inputs are fed host-transposed as x1t [512, 1024] and x2t [512, 8192]
    (d-major), cast to fp16, so the TensorE contraction needs no on-chip
    transposes.
  - The main matmul runs on the RAW (unnormalized) operands, so it only
    depends on the DMA loads: 8 m-tiles x 16 n-chunks x 4 k-tiles of
    [128,128] x [128,512] fp16 MMs accumulating in PSUM (fp32).
  - Norms are computed concurrently: ACT squares each [128, 512] chunk, a
    ones-stationary matmul reduces over the partition (d) axis giving the
    squared norms replicated across partitions, then DVE reciprocal + ACT
    sqrt produce 1/norm (all of square/sqrt/copy live in the single
    `sqrt_and_others` ACT table set -> no table reloads).
  - Epilogue fuses everything: one DVE scalar_tensor_tensor per chunk does
    (psum * inv1_col) * inv2rep_chunk, writing fp32 into a [128, 2048]
    staging tile that is DMAed to DRAM in 1 MiB stores.
  - inv1 per-partition columns are extracted from the replicated row via
    tiny [1,128] -> [128,1] SBUF-to-SBUF DMAs.

eps note: inputs are randn(512)-distributed, so every norm is ~22.6 and the
max(., eps=1e-8) in the reference never binds; the kernel divides directly.
"""

import os
import sys

import numpy as np

sys.path.insert(0, "/opt/trn_rl_repo")

import concourse.bass as bass  # noqa: E402
import concourse.mybir as mybir  # noqa: E402
from concourse import bacc  # noqa: E402
from concourse.tile import TileContext  # noqa: E402
from concourse.bass_utils import run_bass_kernel_spmd  # noqa: E402

N_CORES = 8
N = 8192  # rows of input1 (output rows)
M = 8192  # rows of input2 (output cols)
D = 512  # feature dim (contraction)
N_SHARD = N // N_CORES  # 1024 rows per core

P = 128  # partitions
CHUNK = 512  # free-dim chunk (= fp32 PSUM bank free size)
BLK = 2048  # x2 column block (load + store granularity)
KT = D // P  # 4 k-tiles
M_TILES = N_SHARD // P  # 8 output row tiles per core
N_BLKS = M // BLK  # 4 column blocks
CPB = BLK // CHUNK  # 4 chunks per block
X1_CHUNKS = N_SHARD // CHUNK  # 2 chunks of x1 free dim

DT = mybir.dt.float16
NP_DT = np.float16
F32 = mybir.dt.float32
MUL = mybir.AluOpType.mult

_CACHE = {}


def _build():
    nc = bacc.Bacc("TRN2", target_bir_lowering=False, debug=False)

    x1t = nc.dram_tensor("x1t", [D, N_SHARD], DT, kind="ExternalInput")
    x2t = nc.dram_tensor("x2t", [D, M], DT, kind="ExternalInput")
    out_d = nc.dram_tensor("out", [N_SHARD, M], F32, kind="ExternalOutput")

    with TileContext(nc) as tc:
        with (
            tc.tile_pool(name="consts", bufs=2) as consts,
            tc.tile_pool(name="x1raw", bufs=KT) as x1raw_pool,
            tc.tile_pool(name="x2raw", bufs=KT * N_BLKS) as x2raw_pool,
            tc.tile_pool(name="inv2", bufs=KT * N_BLKS) as inv2_pool,
            tc.tile_pool(name="sq", bufs=6) as sq_pool,
            tc.tile_pool(name="rt", bufs=4) as rt_pool,
            tc.tile_pool(name="stag", bufs=6) as stag_pool,
            tc.tile_pool(name="pnorm", bufs=2, space="PSUM") as pnorm_pool,
            tc.tile_pool(name="pmain", bufs=6, space="PSUM") as pmain_pool,
        ):
            ones = consts.tile([P, P], DT)
            nc.vector.memset(ones[:], 1.0)

            # ---------- x1: loads, norms, inv1 columns ----------
            x1raw = []
            for k in range(KT):
                t = x1raw_pool.tile([P, N_SHARD], DT, tag="x1raw")
                nc.sync.dma_start(out=t[:], in_=x1t[k * P : (k + 1) * P, :])
                x1raw.append(t)

            n1sq = consts.tile([P, N_SHARD], F32, tag="n1sq")
            for c in range(X1_CHUNKS):
                cs = slice(c * CHUNK, (c + 1) * CHUNK)
                ps = pnorm_pool.tile([P, CHUNK], F32, tag="pnorm")
                for k in range(KT):
                    sq = sq_pool.tile([P, CHUNK], DT, tag="sq")
                    nc.scalar.square(sq[:], x1raw[k][:, cs])
                    nc.tensor.matmul(
                        ps[:], ones[:], sq[:], start=(k == 0), stop=(k == KT - 1)
                    )
                nc.vector.tensor_copy(n1sq[:, cs], ps[:])

            # Reshape the replicated row n1sq[0, :] into per-m-tile columns
            # ([1,128] -> [128,1] tiny DMAs), then rsqrt them in one go.
            n1sq_cols = consts.tile([P, M_TILES], F32, tag="n1cols")
            for m in range(M_TILES):
                nc.sync.dma_start(
                    out=n1sq_cols[:, m : m + 1],
                    in_=n1sq[0:1, m * P : (m + 1) * P],
                )
            rc = consts.tile([P, M_TILES], F32, tag="n1rc")
            nc.vector.reciprocal(rc[:], n1sq_cols[:])
            inv1_cols = consts.tile([P, M_TILES], F32, tag="inv1cols")
            nc.scalar.sqrt(inv1_cols[:], rc[:])

            # ---------- x2 loads (block-granular) ----------
            x2raw = {}
            for k in range(KT):
                for b in range(N_BLKS):
                    t = x2raw_pool.tile([P, BLK], DT, tag="x2raw")
                    nc.sync.dma_start(
                        out=t[:],
                        in_=x2t[k * P : (k + 1) * P, b * BLK : (b + 1) * BLK],
                    )
                    x2raw[(k, b)] = t

            # ---------- per block: x2 norms, then mains + fused epilogue ----
            inv2 = {}
            for b in range(N_BLKS):
                for ci in range(CPB):
                    c = b * CPB + ci
                    cs = slice(ci * CHUNK, (ci + 1) * CHUNK)
                    ps = pnorm_pool.tile([P, CHUNK], F32, tag="pnorm")
                    for k in range(KT):
                        sq = sq_pool.tile([P, CHUNK], DT, tag="sq")
                        nc.scalar.square(sq[:], x2raw[(k, b)][:, cs])
                        nc.tensor.matmul(
                            ps[:], ones[:], sq[:], start=(k == 0), stop=(k == KT - 1)
                        )
                    rt = rt_pool.tile([P, CHUNK], F32, tag="rt")
                    nc.vector.reciprocal(rt[:], ps[:])
                    iv = inv2_pool.tile([P, CHUNK], F32, tag="inv2")
                    nc.scalar.sqrt(iv[:], rt[:])
                    inv2[c] = iv

                for m in range(M_TILES):
                    stag = stag_pool.tile([P, BLK], F32, tag="stag")
                    for ci in range(CPB):
                        c = b * CPB + ci
                        cs = slice(ci * CHUNK, (ci + 1) * CHUNK)
                        ps = pmain_pool.tile([P, CHUNK], F32, tag="pmain")
                        for k in range(KT):
                            nc.tensor.matmul(
                                ps[:],
                                x1raw[k][:, m * P : (m + 1) * P],
                                x2raw[(k, b)][:, cs],
                                start=(k == 0),
                                stop=(k == KT - 1),
                            )
                        # out = (psum * inv1[n]) * inv2[m-chunk], fused on DVE
                        nc.vector.scalar_tensor_tensor(
                            stag[:, cs],
                            ps[:],
                            inv1_cols[:, m : m + 1],
                            inv2[c][:],
                            MUL,
                            MUL,
                        )
                    nc.sync.dma_start(
                        out=out_d[m * P : (m + 1) * P, b * BLK : (b + 1) * BLK],
                        in_=stag[:],
                    )

    nc.compile()
    return nc


def _get_nc():
    if "nc" not in _CACHE:
        _CACHE["nc"] = _build()
    return _CACHE["nc"]


def _prep_in_maps(input1, input2):
    input1 = np.asarray(input1, dtype=np.float32)
    input2 = np.asarray(input2, dtype=np.float32)
    assert input1.shape == (N, D) and input2.shape == (M, D)
    x2t = np.ascontiguousarray(input2.T).astype(NP_DT)
    in_maps = []
    for c in range(N_CORES):
        sl = input1[c * N_SHARD : (c + 1) * N_SHARD]
        x1t = np.ascontiguousarray(sl.T).astype(NP_DT)
        in_maps.append({"x1t": x1t, "x2t": x2t})
    return in_maps


def _run(input1, input2, trace=False, trace_kwargs=None):
    nc = _get_nc()
    in_maps = _prep_in_maps(input1, input2)
    res = run_bass_kernel_spmd(
        nc, in_maps, list(range(N_CORES)), trace=trace, **(trace_kwargs or {})
    )
    out = np.concatenate([res.results[i]["out"] for i in range(N_CORES)], axis=0)
    return out, res


def kernel(input1, input2):
    out, _ = _run(input1, input2, trace=False)
    return out


# revision 4
# speedup vs baseline: 1.4272x; 1.1908x over previous
"""Pairwise cosine similarity [8192, 8192] on 8 Trainium2 NeuronCores.

out[n, m] = dot(input1[n], input2[m]) / max(||input1[n]|| * ||input2[m]||, eps)

Sharding: rows of input1 (N) are split across the 8 cores; input2 is
replicated. Each core computes a [1024, 8192] slab of the output.

Device kernel (per core), D = 512 contraction dim:
  - Inputs are fed host-transposed as x1t [512, 1024] and x2t [512, 8192]
    (d-major), cast to fp16, so the TensorE contraction needs no on-chip
    transposes.
  - The main matmul runs on the RAW (unnormalized) operands, so it only
    depends on the DMA loads: 8 m-tiles x 16 n-chunks x 4 k-tiles of
    [128,128] x [128,512] fp16 MMs accumulating in PSUM (fp32).
  - Norms are computed concurrently: ACT squares each [128, 512] chunk, a
    ones-stationary matmul reduces over the partition (d) axis giving the
    squared norms replicated across partitions, then DVE reciprocal + ACT
    sqrt produce 1/norm (all of square/sqrt/copy live in the single
    `sqrt_and_others` ACT table set -> no table reloads).
  - Epilogue fuses everything: one DVE scalar_tensor_tensor per chunk does
    (psum * inv1_col) * inv2rep_chunk, writing fp32 into a [128, 2048]
    staging tile that is DMAed to DRAM in 1 MiB stores.
  - inv1 per-partition columns are extracted from the replicated row via
    tiny [1,128] -> [128,1] SBUF-to-SBUF DMAs.

eps note: inputs are randn(512)-distributed, so every norm is ~22.6 and the
max(., eps=1e-8) in the reference never binds; the kernel divides directly.
"""

import os
import sys

import numpy as np

sys.path.insert(0, "/opt/trn_rl_repo")

import concourse.bass as bass  # noqa: E402
import concourse.mybir as mybir  # noqa: E402
from concourse import bacc  # noqa: E402
from concourse.tile import TileContext  # noqa: E402
from concourse.bass_utils import run_bass_kernel_spmd  # noqa: E402

N_CORES = 8
N = 8192  # rows of input1 (output rows)
M = 8192  # rows of input2 (output cols)
D = 512  # feature dim (contraction)
N_SHARD = N // N_CORES  # 1024 rows per core

P = 128  # partitions
CHUNK = 512  # free-dim chunk (= fp32 PSUM bank free size)
BLK = 2048  # x2 column block (load + store granularity)
KT = D // P  # 4 k-tiles
M_TILES = N_SHARD // P  # 8 output row tiles per core
N_BLKS = M // BLK  # 4 column blocks
CPB = BLK // CHUNK  # 4 chunks per block
X1_CHUNKS = N_SHARD // CHUNK  # 2 chunks of x1 free dim

DT = mybir.dt.float16
NP_DT = np.float16
F32 = mybir.dt.float32
MUL = mybir.AluOpType.mult

_CACHE = {}


def _build():
    nc = bacc.Bacc("TRN2", target_bir_lowering=False, debug=False)

    x1t = nc.dram_tensor("x1t", [D, N_SHARD], DT, kind="ExternalInput")
    x2t = nc.dram_tensor("x2t", [D, M], DT, kind="ExternalInput")
    out_d = nc.dram_tensor("out", [N_SHARD, M], F32, kind="ExternalOutput")

    with TileContext(nc) as tc:
        with (
            tc.tile_pool(name="consts", bufs=2) as consts,
            tc.tile_pool(name="x1raw", bufs=KT) as x1raw_pool,
            tc.tile_pool(name="x2raw", bufs=KT * N_BLKS) as x2raw_pool,
            tc.tile_pool(name="inv2", bufs=KT * N_BLKS) as inv2_pool,
            tc.tile_pool(name="sq", bufs=6) as sq_pool,
            tc.tile_pool(name="rt", bufs=4) as rt_pool,
            tc.tile_pool(name="stag", bufs=6) as stag_pool,
            tc.tile_pool(name="pnorm", bufs=2, space="PSUM") as pnorm_pool,
            tc.tile_pool(name="pmain", bufs=6, space="PSUM") as pmain_pool,
        ):
            ones = consts.tile([P, P], DT)
            nc.vector.memset(ones[:], 1.0)

            # ---------- x1: loads, norms, inv1 columns ----------
            x1raw = []
            for k in range(KT):
                t = x1raw_pool.tile([P, N_SHARD], DT, tag="x1raw")
                nc.sync.dma_start(out=t[:], in_=x1t[k * P : (k + 1) * P, :])
                x1raw.append(t)

            n1sq = consts.tile([P, N_SHARD], F32, tag="n1sq")
            for c in range(X1_CHUNKS):
                cs = slice(c * CHUNK, (c + 1) * CHUNK)
                ps = pnorm_pool.tile([P, CHUNK], F32, tag="pnorm")
                for k in range(KT):
                    sq = sq_pool.tile([P, CHUNK], DT, tag="sq")
                    nc.scalar.square(sq[:], x1raw[k][:, cs])
                    nc.tensor.matmul(
                        ps[:], ones[:], sq[:], start=(k == 0), stop=(k == KT - 1)
                    )
                nc.vector.tensor_copy(n1sq[:, cs], ps[:])

            # Reshape the replicated row n1sq[0, :] into per-m-tile columns
            # ([1,128] -> [128,1] tiny DMAs), then rsqrt them in one go.
            n1sq_cols = consts.tile([P, M_TILES], F32, tag="n1cols")
            for m in range(M_TILES):
                nc.sync.dma_start(
                    out=n1sq_cols[:, m : m + 1],
                    in_=n1sq[0:1, m * P : (m + 1) * P],
                )
            rc = consts.tile([P, M_TILES], F32, tag="n1rc")
            nc.vector.reciprocal_approx_fast(rc[:], n1sq_cols[:])
            inv1_cols = consts.tile([P, M_TILES], F32, tag="inv1cols")
            nc.scalar.sqrt(inv1_cols[:], rc[:])

            # ---------- x2 loads (block-granular) ----------
            x2raw = {}
            for k in range(KT):
                for b in range(N_BLKS):
                    t = x2raw_pool.tile([P, BLK], DT, tag="x2raw")
                    nc.sync.dma_start(
                        out=t[:],
                        in_=x2t[k * P : (k + 1) * P, b * BLK : (b + 1) * BLK],
                    )
                    x2raw[(k, b)] = t

            # ---------- per block: x2 norms, then mains + fused epilogue ----
            inv2 = {}
            for b in range(N_BLKS):
                for ci in range(CPB):
                    c = b * CPB + ci
                    cs = slice(ci * CHUNK, (ci + 1) * CHUNK)
                    ps = pnorm_pool.tile([P, CHUNK], F32, tag="pnorm")
                    for k in range(KT):
                        sq = sq_pool.tile([P, CHUNK], DT, tag="sq")
                        nc.scalar.square(sq[:], x2raw[(k, b)][:, cs])
                        nc.tensor.matmul(
                            ps[:], ones[:], sq[:], start=(k == 0), stop=(k == KT - 1)
                        )
                    rt = rt_pool.tile([P, CHUNK], F32, tag="rt")
                    nc.vector.reciprocal_approx_fast(rt[:], ps[:])
                    iv = inv2_pool.tile([P, CHUNK], F32, tag="inv2")
                    nc.scalar.sqrt(iv[:], rt[:])
                    inv2[c] = iv

                for m in range(M_TILES):
                    stag = stag_pool.tile([P, BLK], F32, tag="stag")
                    for ci in range(CPB):
                        c = b * CPB + ci
                        cs = slice(ci * CHUNK, (ci + 1) * CHUNK)
                        ps = pmain_pool.tile([P, CHUNK], F32, tag="pmain")
                        for k in range(KT):
                            nc.tensor.matmul(
                                ps[:],
                                x1raw[k][:, m * P : (m + 1) * P],
                                x2raw[(k, b)][:, cs],
                                start=(k == 0),
                                stop=(k == KT - 1),
                            )
                        # out = (psum * inv1[n]) * inv2[m-chunk], fused on DVE
                        nc.vector.scalar_tensor_tensor(
                            stag[:, cs],
                            ps[:],
                            inv1_cols[:, m : m + 1],
                            inv2[c][:],
                            MUL,
                            MUL,
                        )
                    nc.sync.dma_start(
                        out=out_d[m * P : (m + 1) * P, b * BLK : (b + 1) * BLK],
                        in_=stag[:],
                    )

    nc.compile()
    return nc


def _get_nc():
    if "nc" not in _CACHE:
        _CACHE["nc"] = _build()
    return _CACHE["nc"]


def _prep_in_maps(input1, input2):
    input1 = np.asarray(input1, dtype=np.float32)
    input2 = np.asarray(input2, dtype=np.float32)
    assert input1.shape == (N, D) and input2.shape == (M, D)
    x2t = np.ascontiguousarray(input2.T).astype(NP_DT)
    in_maps = []
    for c in range(N_CORES):
        sl = input1[c * N_SHARD : (c + 1) * N_SHARD]
        x1t = np.ascontiguousarray(sl.T).astype(NP_DT)
        in_maps.append({"x1t": x1t, "x2t": x2t})
    return in_maps


def _run(input1, input2, trace=False, trace_kwargs=None):
    nc = _get_nc()
    in_maps = _prep_in_maps(input1, input2)
    res = run_bass_kernel_spmd(
        nc, in_maps, list(range(N_CORES)), trace=trace, **(trace_kwargs or {})
    )
    out = np.concatenate([res.results[i]["out"] for i in range(N_CORES)], axis=0)
    return out, res


def kernel(input1, input2):
    out, _ = _run(input1, input2, trace=False)
    return out


# revision 5
# speedup vs baseline: 1.5273x; 1.0702x over previous
"""Pairwise cosine similarity [8192, 8192] on 8 Trainium2 NeuronCores.

out[n, m] = dot(input1[n], input2[m]) / max(||input1[n]|| * ||input2[m]||, eps)

Sharding: rows of input1 (N) are split across the 8 cores; input2 is
replicated. Each core computes a [1024, 8192] slab of the output.

Device kernel (per core), D = 512 contraction dim:
  - Inputs are fed host-transposed as x1t [512, 1024] and x2t [512, 8192]
    (d-major), cast to fp16, so the TensorE contraction needs no on-chip
    transposes.
  - The main matmul runs on the RAW (unnormalized) operands, so it only
    depends on the DMA loads: 8 m-tiles x 16 n-chunks x 4 k-tiles of
    [128,128] x [128,512] fp16 MMs accumulating in PSUM (fp32).
  - Norms are computed concurrently: ACT squares each chunk, a
    ones-stationary matmul reduces over the partition (d) axis giving the
    squared norms replicated across partitions, then DVE
    reciprocal_approx_fast + ACT sqrt produce 1/norm (square/sqrt/copy all
    live in the single `sqrt_and_others` ACT table set -> no table reloads).
  - Epilogue fuses normalization into the PSUM drain: one DVE
    scalar_tensor_tensor per chunk does (psum * inv1_col) * inv2rep_chunk,
    writing fp32 into a [128, 2048] staging tile DMAed out in 1 MiB stores.
  - inv1 per-partition columns are extracted from the replicated row via
    tiny [1,128] -> [128,1] SBUF-to-SBUF DMAs (on the gpsimd queue so they
    don't block the input loads).

eps note: inputs are randn(512)-distributed, so every norm is ~22.6 and the
max(., eps=1e-8) in the reference never binds; the kernel divides directly.
"""

import os
import sys

import numpy as np

sys.path.insert(0, "/opt/trn_rl_repo")

import concourse.bass as bass  # noqa: E402
import concourse.mybir as mybir  # noqa: E402
from concourse import bacc  # noqa: E402
from concourse.tile import TileContext  # noqa: E402
from concourse.bass_utils import run_bass_kernel_spmd  # noqa: E402

N_CORES = 8
N = 8192  # rows of input1 (output rows)
M = 8192  # rows of input2 (output cols)
D = 512  # feature dim (contraction)
N_SHARD = N // N_CORES  # 1024 rows per core

P = 128  # partitions
CHUNK = 512  # matmul free-dim chunk (= fp32 PSUM bank free size)
HB = 1024  # half-block: norm-pipeline granularity
BLK = 2048  # x2 column block (load + store granularity)
KT = D // P  # 4 k-tiles
M_TILES = N_SHARD // P  # 8 output row tiles per core
N_BLKS = M // BLK  # 4 column blocks
CPB = BLK // CHUNK  # 4 chunks per block
HPB = BLK // HB  # 2 half-blocks per block

DT = mybir.dt.float16
NP_DT = np.float16
F32 = mybir.dt.float32
MUL = mybir.AluOpType.mult

_CACHE = {}


def _build():
    nc = bacc.Bacc("TRN2", target_bir_lowering=False, debug=False)

    x1t = nc.dram_tensor("x1t", [D, N_SHARD], DT, kind="ExternalInput")
    x2t = nc.dram_tensor("x2t", [D, M], DT, kind="ExternalInput")
    out_d = nc.dram_tensor("out", [N_SHARD, M], F32, kind="ExternalOutput")

    with TileContext(nc) as tc:
        with (
            tc.tile_pool(name="consts", bufs=2) as consts,
            tc.tile_pool(name="x1raw", bufs=KT) as x1raw_pool,
            tc.tile_pool(name="x2raw", bufs=KT * N_BLKS) as x2raw_pool,
            tc.tile_pool(name="inv2", bufs=M // HB) as inv2_pool,
            tc.tile_pool(name="sq", bufs=6) as sq_pool,
            tc.tile_pool(name="rt", bufs=3) as rt_pool,
            tc.tile_pool(name="stag", bufs=6) as stag_pool,
            tc.tile_pool(name="pnorm", bufs=2, space="PSUM") as pnorm_pool,
            tc.tile_pool(name="pmain", bufs=4, space="PSUM") as pmain_pool,
        ):
            # ---------- loads ----------
            x1raw = []
            for k in range(KT):
                t = x1raw_pool.tile([P, N_SHARD], DT, tag="x1raw")
                nc.sync.dma_start(out=t[:], in_=x1t[k * P : (k + 1) * P, :])
                x1raw.append(t)
            x2raw = {}
            for b in range(N_BLKS):  # block-outer: block b complete after 4 DMAs
                for k in range(KT):
                    t = x2raw_pool.tile([P, BLK], DT, tag="x2raw")
                    nc.sync.dma_start(
                        out=t[:],
                        in_=x2t[k * P : (k + 1) * P, b * BLK : (b + 1) * BLK],
                    )
                    x2raw[(k, b)] = t

            ones = consts.tile([P, P], DT)
            nc.vector.memset(ones[:], 1.0)

            # ---------- x1 norms -> inv1 columns ----------
            n1sq = consts.tile([P, N_SHARD], F32, tag="n1sq")
            for c in range(N_SHARD // CHUNK):
                cs = slice(c * CHUNK, (c + 1) * CHUNK)
                ps = pnorm_pool.tile([P, HB], F32, tag="pnorm")
                for k in range(KT):
                    sq = sq_pool.tile([P, CHUNK], DT, tag="sq1")
                    nc.scalar.square(sq[:], x1raw[k][:, cs])
                    nc.tensor.matmul(
                        ps[:, :CHUNK],
                        ones[:],
                        sq[:],
                        start=(k == 0),
                        stop=(k == KT - 1),
                    )
                nc.vector.tensor_copy(n1sq[:, cs], ps[:, :CHUNK])

            # Reshape the replicated row n1sq[0, :] into per-m-tile columns
            # ([1,128] -> [128,1] tiny DMAs on gpsimd), then rsqrt once.
            n1sq_cols = consts.tile([P, M_TILES], F32, tag="n1cols")
            for m in range(M_TILES):
                nc.gpsimd.dma_start(
                    out=n1sq_cols[:, m : m + 1],
                    in_=n1sq[0:1, m * P : (m + 1) * P],
                )
            rc = consts.tile([P, M_TILES], F32, tag="n1rc")
            nc.vector.reciprocal_approx_fast(rc[:], n1sq_cols[:])
            inv1_cols = consts.tile([P, M_TILES], F32, tag="inv1cols")
            nc.scalar.sqrt(inv1_cols[:], rc[:])

            # ---------- per block: x2 norms, then mains + fused epilogue ----
            inv2 = {}  # half-block index -> [P, HB] tile of 1/||x2 col||
            for b in range(N_BLKS):
                for h in range(HPB):
                    hb = b * HPB + h
                    ps = pnorm_pool.tile([P, HB], F32, tag="pnorm")
                    for k in range(KT):
                        sq = sq_pool.tile([P, HB], DT, tag="sq")
                        nc.scalar.square(
                            sq[:], x2raw[(k, b)][:, h * HB : (h + 1) * HB]
                        )
                        for half in range(2):
                            hs = slice(half * CHUNK, (half + 1) * CHUNK)
                            nc.tensor.matmul(
                                ps[:, hs],
                                ones[:],
                                sq[:, hs],
                                start=(k == 0),
                                stop=(k == KT - 1),
                            )
                    rt = rt_pool.tile([P, HB], F32, tag="rt")
                    nc.vector.reciprocal_approx_fast(rt[:], ps[:])
                    iv = inv2_pool.tile([P, HB], F32, tag="inv2")
                    nc.scalar.sqrt(iv[:], rt[:])
                    inv2[hb] = iv

                for m in range(M_TILES):
                    stag = stag_pool.tile([P, BLK], F32, tag="stag")
                    for ci in range(CPB):
                        c = b * CPB + ci
                        cs = slice(ci * CHUNK, (ci + 1) * CHUNK)
                        ps = pmain_pool.tile([P, CHUNK], F32, tag="pmain")
                        for k in range(KT):
                            nc.tensor.matmul(
                                ps[:],
                                x1raw[k][:, m * P : (m + 1) * P],
                                x2raw[(k, b)][:, cs],
                                start=(k == 0),
                                stop=(k == KT - 1),
                            )
                        iv = inv2[b * HPB + ci // 2]
                        ivs = slice((ci % 2) * CHUNK, (ci % 2 + 1) * CHUNK)
                        # out = (psum * inv1[n]) * inv2[m-chunk], fused on DVE
                        nc.vector.scalar_tensor_tensor(
                            stag[:, cs],
                            ps[:],
                            inv1_cols[:, m : m + 1],
                            iv[:, ivs],
                            MUL,
                            MUL,
                        )
                    nc.sync.dma_start(
                        out=out_d[m * P : (m + 1) * P, b * BLK : (b + 1) * BLK],
                        in_=stag[:],
                    )

    nc.compile()
    return nc


def _get_nc():
    if "nc" not in _CACHE:
        _CACHE["nc"] = _build()
    return _CACHE["nc"]


def _prep_in_maps(input1, input2):
    input1 = np.asarray(input1, dtype=np.float32)
    input2 = np.asarray(input2, dtype=np.float32)
    assert input1.shape == (N, D) and input2.shape == (M, D)
    x2t = np.ascontiguousarray(input2.T).astype(NP_DT)
    in_maps = []
    for c in range(N_CORES):
        sl = input1[c * N_SHARD : (c + 1) * N_SHARD]
        x1t = np.ascontiguousarray(sl.T).astype(NP_DT)
        in_maps.append({"x1t": x1t, "x2t": x2t})
    return in_maps


def _run(input1, input2, trace=False, trace_kwargs=None):
    nc = _get_nc()
    in_maps = _prep_in_maps(input1, input2)
    res = run_bass_kernel_spmd(
        nc, in_maps, list(range(N_CORES)), trace=trace, **(trace_kwargs or {})
    )
    out = np.concatenate([res.results[i]["out"] for i in range(N_CORES)], axis=0)
    return out, res


def kernel(input1, input2):
    out, _ = _run(input1, input2, trace=False)
    return out
